# revision 1
# baseline (speedup 1.0000x reference)
"""MoE layer (top-2 of 8 experts) Trainium2 Bass kernel.

Strategy: data-parallel over tokens across 8 NeuronCores (2048 tokens/core),
expert weights replicated (8.4 MB).  Per core, a dense all-expert FFN runs in
float32r (full PE rate); the top-2 routing mask is computed on-device in exact
fp32 and folded into the hidden activations before the second matmul, so the
expert combine happens for free in PSUM accumulation.

Dataflow per 512-token tile (feature-major layout, tokens on the free dim):
  gate   : psum_g[8,512]  = Wg.T @ X.T            (fp32, exact)
  topk   : transpose -> max8 -> threshold -> 0/1 mask -> transpose back
  ffn    : psum_h[h,512]  = W1c.T @ X.T           (f32r)
           h_sb = gelu(psum_h + b1) * maskrep     (ACT + DVE)
           psum_y[d,512] += W2c.T @ h_sb          (f32r, accumulated over e,hc)
           psum_y starts from b2 x maskT (tiny K=8 matmul)
  out    : PE-transpose Y.T -> Y, DMA out
"""

import sys

sys.path.insert(0, "/opt/trn_rl_repo")

from contextlib import ExitStack

import numpy as np

import concourse.bacc as bacc
import concourse.bass as bass
import concourse.mybir as mybir
import concourse.tile as tile
from concourse import bass_utils
from concourse.masks import make_identity

N_CORES = 8
B, S, D, E, H = 4, 4096, 256, 8, 512
T = B * S                      # 16384 tokens total
TC = T // N_CORES              # 2048 tokens per core
TILE = 512                     # tokens per tile
NTILES = TC // TILE            # 4
DC = D // 128                  # 2 d-chunks
HC = H // 128                  # 4 h-chunks

F32 = mybir.dt.float32
F32R = mybir.dt.float32r
GELU = mybir.ActivationFunctionType.Gelu
IDENT = mybir.ActivationFunctionType.Identity


def _emit(tc: tile.TileContext, ctx: ExitStack, t_in: dict, t_out):
    nc = tc.nc
    x_d, wg_d, bg_d, w1_d, b1_d, w2_d, b2_d = (
        t_in["x"], t_in["Wg"], t_in["bg"], t_in["W1"], t_in["b1"], t_in["W2"],
        t_in["b2"],
    )
    y_d = t_out

    singles = ctx.enter_context(tc.tile_pool(name="singles", bufs=1))
    xpool = ctx.enter_context(tc.tile_pool(name="xpool", bufs=2))
    xtpool = ctx.enter_context(tc.tile_pool(name="xtpool", bufs=3))
    gpool = ctx.enter_context(tc.tile_pool(name="gpool", bufs=4))
    mpool = ctx.enter_context(tc.tile_pool(name="mpool", bufs=NTILES))
    hpool = ctx.enter_context(tc.tile_pool(name="hpool", bufs=12))
    mrpool = ctx.enter_context(tc.tile_pool(name="mrpool", bufs=2))
    opool = ctx.enter_context(tc.tile_pool(name="opool", bufs=2))
    ps_h = ctx.enter_context(tc.tile_pool(name="ps_h", bufs=4, space="PSUM"))
    ps_m = ctx.enter_context(tc.tile_pool(name="ps_m", bufs=2, space="PSUM"))
    ps_y = ctx.enter_context(tc.tile_pool(name="ps_y", bufs=1, space="PSUM"))

    # ---- persistent SBUF: weights, biases, identity --------------------
    ident = singles.tile([128, 128], F32)
    make_identity(nc, ident[:])

    # W1 [E, D, H] -> per-expert [p(d%128), dc, h]; W2 -> [p(h%128), hc, d].
    # Separate tiles + alternating HWDGE rings so expert e's first matmul
    # only waits for its own 512 KB slice.
    w1_all = singles.tile([128, E, DC, H], F32R)
    w2_all = singles.tile([128, E, HC, D], F32R)
    w1_sb = [w1_all[:, e] for e in range(E)]
    w2_sb = [w2_all[:, e] for e in range(E)]
    # first half of the expert stream on the scalar ring (sync ring starts
    # with the x loads); per-expert 512KB DMAs with 4KB contiguous lines
    for e in range(E // 2):
        nc.scalar.dma_start(out=w1_all[:, e], in_=w1_d[:, e])
        nc.scalar.dma_start(out=w2_all[:, e], in_=w2_d[:, e])
    # small operands off the rings (SWDGE)
    wg_sb = singles.tile([128, DC, E], F32)
    nc.gpsimd.dma_start(out=wg_sb[:], in_=wg_d[:])
    b1_sb = singles.tile([128, E, HC], F32)
    nc.gpsimd.dma_start(out=b1_sb[:], in_=b1_d[:])
    b2_sb = singles.tile([E, D], F32R)
    nc.gpsimd.dma_start(out=b2_sb[:], in_=b2_d[:, :])
    bg_sb = singles.tile([E, 1], F32)
    nc.gpsimd.dma_start(out=bg_sb[:], in_=bg_d[:, None])
    # sel_sb[k, e*128 + m] = 1 if k == e else 0.  lhsT slice [8, 128] at
    # expert e replicates maskT row e across all 128 output partitions.
    sel_sb = singles.tile([E, E * 128], F32R)
    for e in range(E):
        nc.vector.tensor_copy(
            sel_sb[:, e * 128:(e + 1) * 128],
            ident[:E, e:e + 1].to_broadcast([E, 128]),
        )

    # ---- per-tile working set ------------------------------------------
    xt_tiles = []      # X^T  [128(d), dc, 512(tok)] per tile (exact fp32)
    xtr_tiles = []     # X^T rounded to f32r for the FFN matmuls
    mt_tiles = []      # mask^T [8, 512] per tile
    mrep_tiles = []    # mask row e replicated across partitions, per tile
    for t in range(NTILES):
        xt_tiles.append(xtpool.tile([128, DC, TILE], F32, tag="xt", name=f"xt{t}"))
        xtr_tiles.append(xtpool.tile([128, DC, TILE], F32R, tag="xtr", name=f"xtr{t}"))
        mt_tiles.append(mpool.tile([E, TILE], F32R, tag="mt", name=f"mt{t}"))
        mrep_tiles.append(mrpool.tile([128, E, TILE], F32, tag="mrep", name=f"mrep{t}"))

    # ---- phase A: x loads first (sync ring), then w2 stream, then transposes
    x_tiles = []
    for t in range(NTILES):
        t0 = t * TILE
        x_tile = xpool.tile([128, TILE // 128, D], F32, tag="x", bufs=3,
                            name=f"xld{t}")
        nc.sync.dma_start(
            out=x_tile[:],
            in_=x_d[t0:t0 + TILE, :].rearrange("(p cc) d -> p cc d", p=128),
        )
        x_tiles.append(x_tile)
    for e in range(E // 2, E):
        nc.sync.dma_start(out=w1_all[:, e], in_=w1_d[:, e])
        nc.sync.dma_start(out=w2_all[:, e], in_=w2_d[:, e])
    for t in range(NTILES):
        x_tile = x_tiles[t]
        for cc in range(TILE // 128):
            for dc in range(DC):
                ps_t = ps_m.tile([128, 128], F32, tag="pst")
                nc.tensor.transpose(
                    out=ps_t[:],
                    in_=x_tile[:, cc, dc * 128:(dc + 1) * 128],
                    identity=ident[:],
                )
                nc.vector.tensor_copy(
                    xt_tiles[t][:, dc, cc * 128:(cc + 1) * 128], ps_t[:]
                )
        nc.vector.tensor_copy(xtr_tiles[t][:], xt_tiles[t][:])

    # ---- phase B (all tiles): gate + top-2 mask ------------------------
    for t in range(NTILES):
        xt = xt_tiles[t]
        ps_g = ps_h.tile([E, TILE], F32, tag="psh")
        for dc in range(DC):
            nc.tensor.matmul(
                ps_g[:], wg_sb[:, dc, :], xt[:, dc, :],
                start=(dc == 0), stop=(dc == DC - 1),
            )
        g_sb = gpool.tile([E, TILE], F32, tag="gsb")
        nc.scalar.activation(g_sb[:], ps_g[:], IDENT, bias=bg_sb[:, 0:1])

        for cc in range(TILE // 128):
            ps_gt = ps_m.tile([128, E], F32, tag="pst")
            nc.tensor.transpose(
                out=ps_gt[:], in_=g_sb[:, cc * 128:(cc + 1) * 128],
                identity=ident[:E, :E],
            )
            gtok = gpool.tile([128, E], F32, tag="gtok")
            nc.vector.tensor_copy(gtok[:], ps_gt[:])
            m8 = gpool.tile([128, 8], F32, tag="m8")
            nc.vector.max(m8[:], gtok[:])
            mask = gpool.tile([128, E], F32, tag="mask")
            nc.vector.tensor_tensor(
                out=mask[:], in0=gtok[:],
                in1=m8[:, 1:2].to_broadcast([128, E]),
                op=mybir.AluOpType.is_ge,
            )
            ps_mt = ps_m.tile([E, 128], F32, tag="pst")
            nc.tensor.transpose(out=ps_mt[:], in_=mask[:], identity=ident[:])
            nc.vector.tensor_copy(
                mt_tiles[t][:, cc * 128:(cc + 1) * 128], ps_mt[:]
            )
        for e in range(E):
            ps_mr = ps_m.tile([128, TILE], F32, tag="pst")
            nc.tensor.matmul(
                ps_mr[:], sel_sb[:, e * 128:(e + 1) * 128],
                mt_tiles[t][:, :],
                start=True, stop=True,
            )
            nc.vector.tensor_copy(mrep_tiles[t][:, e, :], ps_mr[:])

    # ---- phase C: software-pipelined dense masked FFN ------------------
    # PE executes its stream in order, so the second matmuls of step s-1
    # are emitted AFTER the first matmuls of step s: by the time PE reaches
    # SM(s-1), the gelu+mask chain for its h tiles has had a full step to
    # drain, and PE never stalls on ACT/DVE latency.
    NSTEP = NTILES * E
    h_live = {}

    def emit_fm(t, e):
        xtr = xtr_tiles[t]
        mrep = mrep_tiles[t]
        tiles = []
        for hc in range(HC):
            ps_hh = ps_h.tile([128, TILE], F32, tag="psh",
                              name=f"psh{t}_{e}_{hc}")
            for dc in range(DC):
                nc.tensor.matmul(
                    ps_hh[:],
                    w1_sb[e][:, dc, hc * 128:(hc + 1) * 128],
                    xtr[:, dc, :],
                    start=(dc == 0), stop=(dc == DC - 1),
                )
            h_sb = hpool.tile([128, TILE], F32R, tag="h", name=f"h{t}_{e}_{hc}")
            nc.scalar.activation(
                h_sb[:], ps_hh[:], GELU, bias=b1_sb[:, e, hc:hc + 1]
            )
            eng = nc.vector if hc % 2 == 0 else nc.gpsimd
            eng.tensor_mul(h_sb[:], h_sb[:], mrep[:, e, :])
            tiles.append(h_sb)
        h_live[(t, e)] = tiles

    def emit_b2(t):
        for dc in range(DC):
            nc.tensor.matmul(
                psum_y[t][:, dc, :],
                b2_sb[:, dc * 128:(dc + 1) * 128],
                mt_tiles[t][:, :],
                start=True, stop=False, skip_group_check=True,
            )

    def emit_sm(t, e):
        tiles = h_live.pop((t, e))
        for hc in range(HC):
            for dc in range(DC):
                nc.tensor.matmul(
                    psum_y[t][:, dc, :],
                    w2_sb[e][:, hc, dc * 128:(dc + 1) * 128],
                    tiles[hc][:],
                    start=False,
                    stop=(e == E - 1 and hc == HC - 1 and dc == DC - 1),
                    skip_group_check=True,
                )

    def emit_ycopy(t):
        ysb = opool.tile([128, DC, TILE], F32, tag="ysb", name=f"ysb{t}")
        nc.vector.tensor_copy(ysb[:, 0, :], psum_y[t][:, 0, :])
        nc.scalar.activation(
            ysb[:, 1, :], psum_y[t][:, 1, :],
            mybir.ActivationFunctionType.Copy,
        )
        y_live[t] = ysb

    def emit_out(t):
        t0 = t * TILE
        ysb = y_live.pop(t)
        yt_sb = opool.tile([128, TILE // 128, D], F32, tag="ytsb",
                           name=f"ytsb{t}")
        for cc in range(TILE // 128):
            for dc in range(DC):
                ps_t = ps_m.tile([128, 128], F32, tag="pst",
                                 name=f"pso{t}_{cc}_{dc}")
                nc.tensor.transpose(
                    out=ps_t[:],
                    in_=ysb[:, dc, cc * 128:(cc + 1) * 128],
                    identity=ident[:],
                )
                dst = yt_sb[:, cc, dc * 128:(dc + 1) * 128]
                if (cc * DC + dc) % 2 == 0:
                    nc.vector.tensor_copy(dst, ps_t[:])
                else:
                    nc.scalar.activation(
                        dst, ps_t[:], mybir.ActivationFunctionType.Copy
                    )
        nc.sync.dma_start(
            out=y_d[t0:t0 + TILE, :].rearrange("(p cc) d -> p cc d", p=128),
            in_=yt_sb[:],
        )

    psum_y = {}
    y_live = {}
    for t in range(NTILES):
        psum_y[t] = ps_y.tile([128, DC, TILE], F32, tag="psy",
                              name=f"psy{t}")

    LAG = 2
    for s in range(NSTEP + LAG + 1):
        if s < NSTEP:
            t, e = divmod(s, E)
            emit_fm(t, e)
            if e == LAG:
                emit_b2(t)
        if s >= LAG and s - LAG < NSTEP:
            tp, ep = divmod(s - LAG, E)
            emit_sm(tp, ep)
            if ep == E - 1:
                emit_ycopy(tp)
        if s >= LAG + 1 and s - LAG - 1 < NSTEP:
            tq, eq = divmod(s - LAG - 1, E)
            if eq == E - 1:
                emit_out(tq)

_CACHE = {}

def _build():
    if "nc" in _CACHE:
        return _CACHE["nc"]
    nc = bacc.Bacc("TRN2", target_bir_lowering=False)
    t_in = {
        "x": nc.dram_tensor("x", [TC, D], F32, kind="ExternalInput"),
        "Wg": nc.dram_tensor("Wg", [128, DC, E], F32, kind="ExternalInput"),
        "bg": nc.dram_tensor("bg", [E], F32, kind="ExternalInput"),
        "W1": nc.dram_tensor("W1", [128, E, DC, H], F32R, kind="ExternalInput"),
        "b1": nc.dram_tensor("b1", [128, E, HC], F32, kind="ExternalInput"),
        "W2": nc.dram_tensor("W2", [128, E, HC, D], F32R, kind="ExternalInput"),
        "b2": nc.dram_tensor("b2", [E, D], F32R, kind="ExternalInput"),
    }
    y_d = nc.dram_tensor("y", [TC, D], F32, kind="ExternalOutput")
    with tile.TileContext(nc) as tc:
        with ExitStack() as ctx:
            _emit(tc, ctx, t_in, y_d)
    nc.compile()
    _CACHE["nc"] = nc
    return nc


def _run(inputs: dict, trace: bool = False, **kw):
    nc = _build()
    f = lambda a: np.ascontiguousarray(np.asarray(a, dtype=np.float32))
    x = f(inputs["x"]).reshape(T, D)
    w1 = f(inputs["W1"])
    w2 = f(inputs["W2"])
    wg = f(inputs["Wg"])
    b1 = f(inputs["b1"])
    shared = {
        "Wg": np.ascontiguousarray(wg.reshape(DC, 128, E).transpose(1, 0, 2)),
        "bg": f(inputs["bg"]),
        "W1": np.ascontiguousarray(
            w1.reshape(E, DC, 128, H).transpose(2, 0, 1, 3)),
        "b1": np.ascontiguousarray(
            b1.reshape(E, HC, 128).transpose(2, 0, 1)),
        "W2": np.ascontiguousarray(
            w2.reshape(E, HC, 128, D).transpose(2, 0, 1, 3)),
        "b2": f(inputs["b2"]),
    }
    in_maps = [
        {"x": x[c * TC:(c + 1) * TC], **shared} for c in range(N_CORES)
    ]
    br = bass_utils.run_bass_kernel_spmd(
        nc, in_maps, core_ids=list(range(N_CORES)), trace=trace, **kw
    )
    out = np.concatenate([r["y"] for r in br.results], axis=0)
    return out.reshape(B, S, D), br


def kernel(**inputs) -> np.ndarray:
    out, _ = _run(inputs, trace=False)
    return out



# revision 2
# speedup vs baseline: 1.3732x; 1.3732x over previous
"""MoE layer (top-2 of 8 experts) Trainium2 Bass kernel.

Strategy: data-parallel over tokens across 8 NeuronCores (2048 tokens/core),
expert weights replicated (8.4 MB).  Per core, a dense all-expert FFN runs in
float32r (full PE rate); the top-2 routing mask is computed on-device in exact
fp32 and folded into the hidden activations before the second matmul, so the
expert combine happens for free in PSUM accumulation.

Dataflow per 512-token tile (feature-major layout, tokens on the free dim):
  gate   : psum_g[8,512]  = Wg.T @ X.T            (fp32, exact)
  topk   : transpose -> max8 -> threshold -> 0/1 mask -> transpose back
  ffn    : psum_h[h,512]  = W1c.T @ X.T           (f32r)
           h_sb = gelu(psum_h + b1) * maskrep     (ACT + DVE)
           psum_y[d,512] += W2c.T @ h_sb          (f32r, accumulated over e,hc)
           psum_y starts from b2 x maskT (tiny K=8 matmul)
  out    : PE-transpose Y.T -> Y, DMA out
"""

import sys

sys.path.insert(0, "/opt/trn_rl_repo")

from contextlib import ExitStack

import numpy as np

import concourse.bacc as bacc
import concourse.bass as bass
import concourse.mybir as mybir
import concourse.tile as tile
from concourse import bass_utils
from concourse.masks import make_identity

N_CORES = 8
B, S, D, E, H = 4, 4096, 256, 8, 512
T = B * S                      # 16384 tokens total
TC = T // N_CORES              # 2048 tokens per core
TILE = 512                     # tokens per tile
NTILES = TC // TILE            # 4
DC = D // 128                  # 2 d-chunks
HC = H // 128                  # 4 h-chunks

F32 = mybir.dt.float32
F32R = mybir.dt.float32r
GELU = mybir.ActivationFunctionType.Gelu
IDENT = mybir.ActivationFunctionType.Identity


def _emit(tc: tile.TileContext, ctx: ExitStack, t_in: dict, t_out):
    nc = tc.nc
    x_d, wg_d, bg_d, w1_d, b1_d, w2_d, b2_d = (
        t_in["x"], t_in["Wg"], t_in["bg"], t_in["W1"], t_in["b1"], t_in["W2"],
        t_in["b2"],
    )
    y_d = t_out

    singles = ctx.enter_context(tc.tile_pool(name="singles", bufs=1))
    xpool = ctx.enter_context(tc.tile_pool(name="xpool", bufs=2))
    xtpool = ctx.enter_context(tc.tile_pool(name="xtpool", bufs=3))
    gpool = ctx.enter_context(tc.tile_pool(name="gpool", bufs=4))
    mpool = ctx.enter_context(tc.tile_pool(name="mpool", bufs=NTILES))
    hpool = ctx.enter_context(tc.tile_pool(name="hpool", bufs=12))
    mrpool = ctx.enter_context(tc.tile_pool(name="mrpool", bufs=2))
    opool = ctx.enter_context(tc.tile_pool(name="opool", bufs=2))
    ps_h = ctx.enter_context(tc.tile_pool(name="ps_h", bufs=4, space="PSUM"))
    ps_m = ctx.enter_context(tc.tile_pool(name="ps_m", bufs=2, space="PSUM"))
    ps_y = ctx.enter_context(tc.tile_pool(name="ps_y", bufs=1, space="PSUM"))

    # ---- persistent SBUF: weights, biases, identity --------------------
    ident = singles.tile([128, 128], F32)
    make_identity(nc, ident[:])

    # W1 [E, D, H] -> per-expert [p(d%128), dc, h]; W2 -> [p(h%128), hc, d].
    # Separate tiles + alternating HWDGE rings so expert e's first matmul
    # only waits for its own 512 KB slice.
    w1_all = singles.tile([128, E, DC, H], F32R)
    w2_all = singles.tile([128, E, HC, D], F32R)
    w1_sb = [w1_all[:, e] for e in range(E)]
    w2_sb = [w2_all[:, e] for e in range(E)]
    # first half of the expert stream on the scalar ring (sync ring starts
    # with the x loads); per-expert 512KB DMAs with 4KB contiguous lines
    for e in range(E // 2):
        nc.scalar.dma_start(out=w1_all[:, e], in_=w1_d[:, e])
        nc.scalar.dma_start(out=w2_all[:, e], in_=w2_d[:, e])
    # small operands off the rings (SWDGE)
    wg_sb = singles.tile([128, DC, E], F32)
    nc.gpsimd.dma_start(out=wg_sb[:], in_=wg_d[:])
    b1_sb = singles.tile([128, E, HC], F32)
    nc.gpsimd.dma_start(out=b1_sb[:], in_=b1_d[:])
    b2_sb = singles.tile([E, D], F32R)
    nc.gpsimd.dma_start(out=b2_sb[:], in_=b2_d[:, :])
    bg_sb = singles.tile([E, 1], F32)
    nc.gpsimd.dma_start(out=bg_sb[:], in_=bg_d[:, None])
    # sel_sb[k, e*128 + m] = 1 if k == e else 0.  lhsT slice [8, 128] at
    # expert e replicates maskT row e across all 128 output partitions.
    sel_sb = singles.tile([E, E * 128], F32R)
    for e in range(E):
        nc.vector.tensor_copy(
            sel_sb[:, e * 128:(e + 1) * 128],
            ident[:E, e:e + 1].to_broadcast([E, 128]),
        )

    # ---- per-tile working set ------------------------------------------
    xt_tiles = []      # X^T  [128(d), dc, 512(tok)] per tile (exact fp32)
    xtr_tiles = []     # X^T rounded to f32r for the FFN matmuls
    mt_tiles = []      # mask^T [8, 512] per tile
    mrep_tiles = []    # mask row e replicated across partitions, per tile
    for t in range(NTILES):
        xt_tiles.append(xtpool.tile([128, DC, TILE], F32, tag="xt", name=f"xt{t}"))
        xtr_tiles.append(xtpool.tile([128, DC, TILE], F32R, tag="xtr", name=f"xtr{t}"))
        mt_tiles.append(mpool.tile([E, TILE], F32R, tag="mt", name=f"mt{t}"))
        mrep_tiles.append(mrpool.tile([128, E, TILE], F32, tag="mrep", name=f"mrep{t}"))

    # ---- phase A: x loads first (sync ring), then w2 stream, then transposes
    x_tiles = []
    for t in range(NTILES):
        t0 = t * TILE
        x_tile = xpool.tile([128, TILE // 128, D], F32, tag="x", bufs=3,
                            name=f"xld{t}")
        nc.sync.dma_start(
            out=x_tile[:],
            in_=x_d[t0:t0 + TILE, :].rearrange("(p cc) d -> p cc d", p=128),
        )
        x_tiles.append(x_tile)
    for e in range(E // 2, E):
        nc.sync.dma_start(out=w1_all[:, e], in_=w1_d[:, e])
        nc.sync.dma_start(out=w2_all[:, e], in_=w2_d[:, e])
    for t in range(NTILES):
        x_tile = x_tiles[t]
        for cc in range(TILE // 128):
            for dc in range(DC):
                ps_t = ps_m.tile([128, 128], F32, tag="pst")
                nc.tensor.transpose(
                    out=ps_t[:],
                    in_=x_tile[:, cc, dc * 128:(dc + 1) * 128],
                    identity=ident[:],
                )
                nc.vector.tensor_copy(
                    xt_tiles[t][:, dc, cc * 128:(cc + 1) * 128], ps_t[:]
                )
        nc.vector.tensor_copy(xtr_tiles[t][:], xt_tiles[t][:])

    # ---- phase B (all tiles): gate + top-2 mask ------------------------
    for t in range(NTILES):
        xt = xt_tiles[t]
        ps_g = ps_h.tile([E, TILE], F32, tag="psh")
        for dc in range(DC):
            nc.tensor.matmul(
                ps_g[:], wg_sb[:, dc, :], xt[:, dc, :],
                start=(dc == 0), stop=(dc == DC - 1),
            )
        g_sb = gpool.tile([E, TILE], F32, tag="gsb")
        nc.scalar.activation(g_sb[:], ps_g[:], IDENT, bias=bg_sb[:, 0:1])

        for cc in range(TILE // 128):
            ps_gt = ps_m.tile([128, E], F32, tag="pst")
            nc.tensor.transpose(
                out=ps_gt[:], in_=g_sb[:, cc * 128:(cc + 1) * 128],
                identity=ident[:E, :E],
            )
            gtok = gpool.tile([128, E], F32, tag="gtok")
            nc.vector.tensor_copy(gtok[:], ps_gt[:])
            m8 = gpool.tile([128, 8], F32, tag="m8")
            nc.vector.max(m8[:], gtok[:])
            mask = gpool.tile([128, E], F32, tag="mask")
            nc.vector.tensor_tensor(
                out=mask[:], in0=gtok[:],
                in1=m8[:, 1:2].to_broadcast([128, E]),
                op=mybir.AluOpType.is_ge,
            )
            ps_mt = ps_m.tile([E, 128], F32, tag="pst")
            nc.tensor.transpose(out=ps_mt[:], in_=mask[:], identity=ident[:])
            nc.vector.tensor_copy(
                mt_tiles[t][:, cc * 128:(cc + 1) * 128], ps_mt[:]
            )
        for e in range(E):
            ps_mr = ps_m.tile([128, TILE], F32, tag="pst")
            nc.tensor.matmul(
                ps_mr[:], sel_sb[:, e * 128:(e + 1) * 128],
                mt_tiles[t][:, :],
                start=True, stop=True,
            )
            nc.vector.tensor_copy(mrep_tiles[t][:, e, :], ps_mr[:])

    # ---- phase C: software-pipelined dense masked FFN ------------------
    # PE executes its stream in order, so the second matmuls of step s-1
    # are emitted AFTER the first matmuls of step s: by the time PE reaches
    # SM(s-1), the gelu+mask chain for its h tiles has had a full step to
    # drain, and PE never stalls on ACT/DVE latency.
    NSTEP = NTILES * E
    h_live = {}

    def emit_fm(t, e):
        xtr = xtr_tiles[t]
        mrep = mrep_tiles[t]
        tiles = []
        for hc in range(HC):
            ps_hh = ps_h.tile([128, TILE], F32, tag="psh",
                              name=f"psh{t}_{e}_{hc}")
            for dc in range(DC):
                nc.tensor.matmul(
                    ps_hh[:],
                    w1_sb[e][:, dc, hc * 128:(hc + 1) * 128],
                    xtr[:, dc, :],
                    start=(dc == 0), stop=(dc == DC - 1),
                )
            h_sb = hpool.tile([128, TILE], F32R, tag="h", name=f"h{t}_{e}_{hc}")
            nc.scalar.activation(
                h_sb[:], ps_hh[:], GELU, bias=b1_sb[:, e, hc:hc + 1]
            )
            eng = nc.vector if hc % 2 == 0 else nc.gpsimd
            eng.tensor_mul(h_sb[:], h_sb[:], mrep[:, e, :])
            tiles.append(h_sb)
        h_live[(t, e)] = tiles

    def emit_b2(t):
        for dc in range(DC):
            nc.tensor.matmul(
                psum_y[t][:, dc, :],
                b2_sb[:, dc * 128:(dc + 1) * 128],
                mt_tiles[t][:, :],
                start=True, stop=False, skip_group_check=True,
            )

    def emit_sm(t, e):
        tiles = h_live.pop((t, e))
        for hc in range(HC):
            for dc in range(DC):
                nc.tensor.matmul(
                    psum_y[t][:, dc, :],
                    w2_sb[e][:, hc, dc * 128:(dc + 1) * 128],
                    tiles[hc][:],
                    start=False,
                    stop=(e == E - 1 and hc == HC - 1 and dc == DC - 1),
                    skip_group_check=True,
                )

    def emit_ycopy(t):
        ysb = opool.tile([128, DC, TILE], F32, tag="ysb", name=f"ysb{t}")
        nc.vector.tensor_copy(ysb[:, 0, :], psum_y[t][:, 0, :])
        nc.scalar.activation(
            ysb[:, 1, :], psum_y[t][:, 1, :],
            mybir.ActivationFunctionType.Copy,
        )
        y_live[t] = ysb

    def emit_out(t):
        t0 = t * TILE
        ysb = y_live.pop(t)
        yt_sb = opool.tile([128, TILE // 128, D], F32, tag="ytsb",
                           name=f"ytsb{t}")
        for cc in range(TILE // 128):
            for dc in range(DC):
                ps_t = ps_m.tile([128, 128], F32, tag="pst",
                                 name=f"pso{t}_{cc}_{dc}")
                nc.tensor.transpose(
                    out=ps_t[:],
                    in_=ysb[:, dc, cc * 128:(cc + 1) * 128],
                    identity=ident[:],
                )
                dst = yt_sb[:, cc, dc * 128:(dc + 1) * 128]
                if (cc * DC + dc) % 2 == 0:
                    nc.vector.tensor_copy(dst, ps_t[:])
                else:
                    nc.scalar.activation(
                        dst, ps_t[:], mybir.ActivationFunctionType.Copy
                    )
        nc.sync.dma_start(
            out=y_d[t0:t0 + TILE, :].rearrange("(p cc) d -> p cc d", p=128),
            in_=yt_sb[:],
        )

    psum_y = {}
    y_live = {}
    for t in range(NTILES):
        psum_y[t] = ps_y.tile([128, DC, TILE], F32, tag="psy",
                              name=f"psy{t}")

    LAG = 3
    for s in range(NSTEP + LAG + 1):
        if s < NSTEP:
            t, e = divmod(s, E)
            emit_fm(t, e)
            if e == LAG:
                emit_b2(t)
        if s >= LAG and s - LAG < NSTEP:
            tp, ep = divmod(s - LAG, E)
            emit_sm(tp, ep)
            if ep == E - 1:
                emit_ycopy(tp)
        if s >= LAG + 1 and s - LAG - 1 < NSTEP:
            tq, eq = divmod(s - LAG - 1, E)
            if eq == E - 1:
                emit_out(tq)

_CACHE = {}

def _build():
    if "nc" in _CACHE:
        return _CACHE["nc"]
    nc = bacc.Bacc("TRN2", target_bir_lowering=False)
    t_in = {
        "x": nc.dram_tensor("x", [TC, D], F32, kind="ExternalInput"),
        "Wg": nc.dram_tensor("Wg", [128, DC, E], F32, kind="ExternalInput"),
        "bg": nc.dram_tensor("bg", [E], F32, kind="ExternalInput"),
        "W1": nc.dram_tensor("W1", [128, E, DC, H], F32R, kind="ExternalInput"),
        "b1": nc.dram_tensor("b1", [128, E, HC], F32, kind="ExternalInput"),
        "W2": nc.dram_tensor("W2", [128, E, HC, D], F32R, kind="ExternalInput"),
        "b2": nc.dram_tensor("b2", [E, D], F32R, kind="ExternalInput"),
    }
    y_d = nc.dram_tensor("y", [TC, D], F32, kind="ExternalOutput")
    with tile.TileContext(nc) as tc:
        with ExitStack() as ctx:
            _emit(tc, ctx, t_in, y_d)
    nc.compile()
    _CACHE["nc"] = nc
    return nc


def _run(inputs: dict, trace: bool = False, **kw):
    nc = _build()
    f = lambda a: np.ascontiguousarray(np.asarray(a, dtype=np.float32))
    x = f(inputs["x"]).reshape(T, D)
    w1 = f(inputs["W1"])
    w2 = f(inputs["W2"])
    wg = f(inputs["Wg"])
    b1 = f(inputs["b1"])
    shared = {
        "Wg": np.ascontiguousarray(wg.reshape(DC, 128, E).transpose(1, 0, 2)),
        "bg": f(inputs["bg"]),
        "W1": np.ascontiguousarray(
            w1.reshape(E, DC, 128, H).transpose(2, 0, 1, 3)),
        "b1": np.ascontiguousarray(
            b1.reshape(E, HC, 128).transpose(2, 0, 1)),
        "W2": np.ascontiguousarray(
            w2.reshape(E, HC, 128, D).transpose(2, 0, 1, 3)),
        "b2": f(inputs["b2"]),
    }
    in_maps = [
        {"x": x[c * TC:(c + 1) * TC], **shared} for c in range(N_CORES)
    ]
    br = bass_utils.run_bass_kernel_spmd(
        nc, in_maps, core_ids=list(range(N_CORES)), trace=trace, **kw
    )
    out = np.concatenate([r["y"] for r in br.results], axis=0)
    return out.reshape(B, S, D), br


def kernel(**inputs) -> np.ndarray:
    out, _ = _run(inputs, trace=False)
    return out

